# revision 11
# baseline (speedup 1.0000x reference)
"""AdaptiveRankLinear on Trainium2, 8-core data-parallel Bass/Tile kernel.

Computes  y = x + gamma * (((rmsnorm(x) * norm_weight) @ U) * (S*keep)) @ V
with keep = adaptive-rank mask from the singular-value energy of S.

Sharding: x is flattened to [8192, 4096] tokens and split into 8 shards of
1024 tokens (one per NeuronCore); U/S/V/norm_weight/gamma are tiny and
replicated (folded host-side into two small matrices).

v4: bf16 I/O; rstd deferred to the final fused (delta*rstd)+x op; 2-tile
super-tiles batch the U-contraction (N=256); V-expansion runs in PE
transpose-mode so delta lands in PSUM as bf16 (2x DVE reads); transposes
write bf16 PSUM banks of 8 blocks.

Per-core device pipeline (per 2-tile super-tile of 256 tokens):
  DMA x(bf16) -> ACT square+accum -> sqrt -> DVE reciprocal -> rstd
  PE transpose-mode x_j.T @ I (bf16 psum, 8 blocks/bank) -> ACT/DVE evac
  PE h += U2_j.T @ xT_j (f32 psum, N=256) -> DVE evac bf16
  PE delta = hs.T @ V2 (transpose-mode, bf16 psum, N=1024)
  DVE y = delta*rstd + x (2x) -> DMA out (bf16).
"""
import ml_dtypes
import numpy as np

import concourse.bass as bass
import concourse.tile as tile
from concourse import mybir
from concourse.bass_utils import run_bass_kernel_spmd
from concourse.vector_clock import ScopedClock

# ----------------------------------------------------------------------------
# Workaround: this container's walrus accepts at most ONE sync wait per
# instruction, while Tile's sem-assigner can attach several.  Split extras
# into engine-local no-ops placed immediately before the over-waited
# instruction; same for the kernel-tail drain.
# ----------------------------------------------------------------------------
_MAXW = 1


def _split_bb_waits(nc, bb):
    insts = list(bb.instructions)
    out = []
    changed = False
    for inst in insts:
        si = inst.sync_info
        if si is not None and len(si.on_wait) > _MAXW:
            changed = True
            waits = list(si.on_wait)
            extra, keep = waits[:-_MAXW], waits[-_MAXW:]
            for k, w in enumerate(extra):
                nop = mybir.InstNoOp(name=f"{inst.name}_wsplit{k}", ins=[],
                                     outs=[])
                nop.engine = inst.engine
                nop.sync_info = mybir.SyncInfo(on_wait=[w], on_update=[])
                nc.register_instruction(nop, overwrite=True)
                out.append(nop)
            inst.sync_info = mybir.SyncInfo(on_wait=keep,
                                            on_update=list(si.on_update))
        out.append(inst)
    if changed:
        bb.instructions = out


def _patched_drain_and_barrier(self, tick_clock, wait_clock):
    for f in self.nc.m.functions:
        for bb in f.blocks:
            _split_bb_waits(self.nc, bb)

    drain_inst = self.nc.sync.drain()
    wait_clock.add_sem_waits(
        drain_inst.ins, ScopedClock({None: tick_clock.global_clock})
    )
    si = drain_inst.ins.sync_info
    if si is not None and len(si.on_wait) > _MAXW:
        waits = list(si.on_wait)
        drain_inst.ins.sync_info = mybir.SyncInfo(
            on_wait=waits[:_MAXW], on_update=list(si.on_update)
        )
        rest = waits[_MAXW:]
        for i in range(0, len(rest), _MAXW):
            nop = self.nc.sync.nop(nofuse=True, hint="drain_wait_spill")
            nop.ins.sync_info = mybir.SyncInfo(
                on_wait=rest[i:i + _MAXW], on_update=[]
            )

    self.nc.all_engine_barrier()
    assert self.sems is not None
    popped = self.nc._tile_sem_poison_stack.pop()
    assert popped is self._sem_poison
    self.nc.clear_and_free_semaphores(list(self.sems.allocated().values()))
    self.nc.all_engine_barrier()


tile.TileContext._drain_and_barrier = _patched_drain_and_barrier

# ----------------------------------------------------------------------------
# Problem constants (hardcoded; kernel.py must be self-contained).
# ----------------------------------------------------------------------------
N_CORES = 8
B, T, D = 4, 2048, 4096
TOK = B * T              # 8192
R = 16
SHARD = TOK // N_CORES   # 1024
PT = 128                 # tokens per tile
KB = D // 128            # 32 contraction blocks
EPS = 1e-6
ENERGY_THRESHOLD = 0.95
F32 = mybir.dt.float32
BF16 = mybir.dt.bfloat16
NP_BF16 = ml_dtypes.bfloat16
AF = mybir.ActivationFunctionType
ALU = mybir.AluOpType

SUP = 2                  # tiles per super-tile (U-contraction batch)
NS = SHARD // (SUP * PT)  # 4 super-tiles per core
TG = 8                   # transpose blocks per PSUM group (bank = 1024 bf16)
NG = KB // TG            # 4 groups per tile
VW = 512                 # V-expansion width (transpose-mode matmul limit)
NV = D // VW             # 8 V-matmuls per tile


def build_nc():
    nc = bass.Bass("TRN2", target_bir_lowering=False, debug=False,
                   num_devices=N_CORES)
    x = nc.declare_dram_parameter("x", [SHARD, D], BF16, isOutput=False)
    u = nc.declare_dram_parameter("u", [128, KB * R], BF16, isOutput=False)
    v = nc.declare_dram_parameter("v", [R, D], BF16, isOutput=False)
    eye = nc.declare_dram_parameter("eye", [PT, PT], BF16, isOutput=False)
    out = nc.declare_dram_parameter("out", [SHARD, D], BF16, isOutput=True)

    with tile.TileContext(nc) as tc:
        with (
            tc.tile_pool(name="singles", bufs=1) as singles,
            tc.tile_pool(name="xin", bufs=2) as xin,
            tc.tile_pool(name="xtp", bufs=2) as xtp,
            tc.tile_pool(name="yout", bufs=3) as yout,
            tc.tile_pool(name="smalls", bufs=8) as smalls,
            tc.tile_pool(name="scratch", bufs=1) as scratch,
            tc.tile_pool(name="xt_ps", bufs=3, space="PSUM") as xt_ps,
            tc.tile_pool(name="h_ps", bufs=2, space="PSUM") as h_ps,
            tc.tile_pool(name="d_ps", bufs=3, space="PSUM") as d_ps,
        ):
            u_sb = singles.tile([128, KB, R], BF16)
            nc.sync.dma_start(out=u_sb, in_=u.rearrange("p (k r) -> p k r", r=R))
            v_sb = singles.tile([R, D], BF16)
            nc.sync.dma_start(out=v_sb, in_=v[:, :])
            eye_sb = singles.tile([PT, PT], BF16)
            nc.sync.dma_start(out=eye_sb, in_=eye[:, :])
            eps_sb = singles.tile([128, 1], F32)
            nc.vector.memset(eps_sb, EPS)

            for s in range(NS):
                s0 = s * SUP * PT
                # x super-tile: [p, ti, d] with token = s0 + ti*PT + p
                x_sup = xin.tile([PT, SUP, D], BF16, tag="x_sup")
                for ti in range(SUP):
                    nc.sync.dma_start(
                        out=x_sup[:, ti, :],
                        in_=x[s0 + ti * PT:s0 + (ti + 1) * PT, :])

                # xT super-layout: [p=dblk, j, ti, t]
                xt_sb = xtp.tile([128, KB, SUP, PT], BF16)
                rstds = []
                for ti in range(SUP):
                    x_t = x_sup[:, ti, :]
                    # RMS stats: sumsq -> rstd (fp32)
                    sumsq = smalls.tile([PT, 1], F32, tag="sumsq")
                    sq_scr = scratch.tile([PT, D], BF16, tag="sq_scr")
                    nc.scalar.activation(out=sq_scr, in_=x_t, func=AF.Square,
                                         accum_out=sumsq)
                    std = smalls.tile([PT, 1], F32, tag="std")
                    nc.scalar.activation(out=std, in_=sumsq, func=AF.Sqrt,
                                         bias=eps_sb, scale=1.0 / D)
                    rstd = smalls.tile([PT, 1], F32, tag="rstd")
                    nc.vector.reciprocal(out=rstd, in_=std)
                    rstds.append(rstd)

                    # PE transpose: xT_j = x_j.T @ I (bf16 psum)
                    for g in range(NG):
                        tp = xt_ps.tile([128, TG * PT], BF16, tag="tp")
                        for q in range(TG):
                            j = g * TG + q
                            nc.tensor.matmul(
                                out=tp[:, q * PT:(q + 1) * PT],
                                lhsT=x_t[:, j * 128:(j + 1) * 128],
                                rhs=eye_sb, is_transpose=True,
                                start=True, stop=True)
                        dst = xt_sb[:, g * TG:(g + 1) * TG, ti, :]
                        if g == 0:
                            nc.scalar.copy(out=dst, in_=tp)
                        else:
                            nc.vector.tensor_copy(out=dst, in_=tp)

                # h[r, (ti,t)] += U2_j.T @ xT_j over 32 blocks (f32 psum)
                h_psum = h_ps.tile([R, SUP * PT], F32, tag="h")
                for j in range(KB):
                    nc.tensor.matmul(
                        out=h_psum,
                        lhsT=u_sb[:, j, :],
                        rhs=xt_sb[:, j, :, :],
                        start=(j == 0), stop=(j == KB - 1))
                hs_sb = smalls.tile([R, SUP * PT], BF16, tag="hs")
                nc.vector.tensor_copy(out=hs_sb, in_=h_psum)

                # delta = hs.T @ V2 (f32 psum, N=VW);
                # y = delta*rstd + x fused on DVE
                for ti in range(SUP):
                    t0 = s0 + ti * PT
                    y_sb = yout.tile([PT, D], BF16)
                    for n in range(NV):
                        dps = d_ps.tile([PT, VW], F32, tag="d")
                        nc.tensor.matmul(
                            out=dps,
                            lhsT=hs_sb[:, ti * PT:(ti + 1) * PT],
                            rhs=v_sb[:, n * VW:(n + 1) * VW],
                            start=True, stop=True)
                        nc.vector.scalar_tensor_tensor(
                            out=y_sb[:, n * VW:(n + 1) * VW],
                            in0=dps, scalar=rstds[ti],
                            in1=x_sup[:, ti, n * VW:(n + 1) * VW],
                            op0=ALU.mult, op1=ALU.add)
                        if n == NV // 2 - 1:
                            nc.gpsimd.dma_start(out=out[t0:t0 + PT, :D // 2],
                                                in_=y_sb[:, :D // 2])
                    nc.gpsimd.dma_start(out=out[t0:t0 + PT, D // 2:],
                                        in_=y_sb[:, D // 2:])
    return nc


def _rank_mask_np(S):
    s_abs = np.abs(S)
    cum = np.cumsum(s_abs) / max(float(s_abs.sum()), 1e-8)
    hit = cum >= ENERGY_THRESHOLD
    r = int(np.argmax(hit)) + 1 if hit.any() else S.shape[0]
    return (np.arange(S.shape[0]) < r).astype(S.dtype)


def make_in_maps(x, U, S, V, norm_weight, gamma):
    S = np.asarray(S, dtype=np.float32)
    keep = _rank_mask_np(S)
    U2 = (np.asarray(norm_weight, dtype=np.float32)[:, None]
          * np.asarray(U, dtype=np.float32)
          * (S * keep)[None, :]).astype(NP_BF16)
    U2 = np.ascontiguousarray(
        U2.reshape(KB, 128, R).transpose(1, 0, 2).reshape(128, KB * R))
    V2 = (np.asarray(V, dtype=np.float32)
          * np.asarray(gamma, dtype=np.float32)[None, :]).astype(NP_BF16)
    eye = np.eye(PT, dtype=NP_BF16)
    xf = np.ascontiguousarray(
        np.asarray(x, dtype=np.float32).reshape(TOK, D)).astype(NP_BF16)
    shards = np.split(xf, N_CORES, axis=0)
    return [{"x": s, "u": U2, "v": V2, "eye": eye} for s in shards]


_CACHED_NC = None


def run(x, U, S, V, norm_weight, gamma, trace=False, **kw):
    global _CACHED_NC
    if _CACHED_NC is None:
        _CACHED_NC = build_nc()
    in_maps = make_in_maps(x, U, S, V, norm_weight, gamma)
    res = run_bass_kernel_spmd(_CACHED_NC, in_maps,
                               core_ids=list(range(N_CORES)), trace=trace,
                               **kw)
    outs = [np.asarray(res.results[i]["out"]) for i in range(N_CORES)]
    y = np.concatenate(outs, axis=0).reshape(B, T, D).astype(np.float32)
    return y, res


def kernel(x, U, S, V, norm_weight, gamma):
    y, _ = run(x, U, S, V, norm_weight, gamma, trace=False)
    return y


# revision 14
# speedup vs baseline: 1.1611x; 1.1611x over previous
"""AdaptiveRankLinear on Trainium2, 8-core data-parallel Bass/Tile kernel.

Computes  y = x + gamma * (((rmsnorm(x) * norm_weight) @ U) * (S*keep)) @ V
with keep = adaptive-rank mask from the singular-value energy of S.

Sharding: x is flattened to [8192, 4096] tokens and split into 8 shards of
1024 tokens (one per NeuronCore); U/S/V/norm_weight/gamma are tiny and
replicated (folded host-side into two small matrices).

v6: bf16 I/O; rstd deferred to the fused (delta*rstd)+x output op; square
pass split ACT/GPSIMD; xT evacs split ACT/DVE (DVE gets 2x on bf16 PSUM);
y-ops split DVE-direct / ACT-copy-scale+DVE-add; V-expansion emitted one
tile late so PE never stalls on the delta PSUM bank rotation.

Per-core device pipeline (per 128-token tile):
  DMA x(bf16) -> ACT+GPS square halves (accum) -> add/sqrt/recip -> rstd
  PE transpose x_j.T @ I (bf16 psum, 8 blocks/bank) -> ACT/DVE evac
  PE hT += U2_j.T @ xT_j (f32 psum) -> DVE evac bf16
  [one tile later] PE delta = hT.T @ V2 (f32 psum)
  DVE/ACT y = delta*rstd + x -> DMA out (bf16).
"""
import ml_dtypes
import numpy as np

import concourse.bass as bass
import concourse.tile as tile
from concourse import mybir
from concourse.bass_utils import run_bass_kernel_spmd
from concourse.vector_clock import ScopedClock

# ----------------------------------------------------------------------------
# Workaround: this container's walrus accepts at most ONE sync wait per
# instruction, while Tile's sem-assigner can attach several.  Split extras
# into engine-local no-ops placed immediately before the over-waited
# instruction; same for the kernel-tail drain.
# ----------------------------------------------------------------------------
_MAXW = 1


def _split_bb_waits(nc, bb):
    insts = list(bb.instructions)
    out = []
    changed = False
    for inst in insts:
        si = inst.sync_info
        if si is not None and len(si.on_wait) > _MAXW:
            changed = True
            waits = list(si.on_wait)
            extra, keep = waits[:-_MAXW], waits[-_MAXW:]
            for k, w in enumerate(extra):
                nop = mybir.InstNoOp(name=f"{inst.name}_wsplit{k}", ins=[],
                                     outs=[])
                nop.engine = inst.engine
                nop.sync_info = mybir.SyncInfo(on_wait=[w], on_update=[])
                nc.register_instruction(nop, overwrite=True)
                out.append(nop)
            inst.sync_info = mybir.SyncInfo(on_wait=keep,
                                            on_update=list(si.on_update))
        out.append(inst)
    if changed:
        bb.instructions = out


def _patched_drain_and_barrier(self, tick_clock, wait_clock):
    for f in self.nc.m.functions:
        for bb in f.blocks:
            _split_bb_waits(self.nc, bb)

    drain_inst = self.nc.sync.drain()
    wait_clock.add_sem_waits(
        drain_inst.ins, ScopedClock({None: tick_clock.global_clock})
    )
    si = drain_inst.ins.sync_info
    if si is not None and len(si.on_wait) > _MAXW:
        waits = list(si.on_wait)
        drain_inst.ins.sync_info = mybir.SyncInfo(
            on_wait=waits[:_MAXW], on_update=list(si.on_update)
        )
        rest = waits[_MAXW:]
        for i in range(0, len(rest), _MAXW):
            nop = self.nc.sync.nop(nofuse=True, hint="drain_wait_spill")
            nop.ins.sync_info = mybir.SyncInfo(
                on_wait=rest[i:i + _MAXW], on_update=[]
            )

    self.nc.all_engine_barrier()
    assert self.sems is not None
    popped = self.nc._tile_sem_poison_stack.pop()
    assert popped is self._sem_poison
    self.nc.clear_and_free_semaphores(list(self.sems.allocated().values()))
    self.nc.all_engine_barrier()


tile.TileContext._drain_and_barrier = _patched_drain_and_barrier

# ----------------------------------------------------------------------------
# Problem constants (hardcoded; kernel.py must be self-contained).
# ----------------------------------------------------------------------------
N_CORES = 8
B, T, D = 4, 2048, 4096
TOK = B * T              # 8192
R = 16
SHARD = TOK // N_CORES   # 1024
PT = 128                 # tokens per tile
NT = SHARD // PT         # 8
KB = D // 128            # 32 contraction blocks
EPS = 1e-6
ENERGY_THRESHOLD = 0.95
F32 = mybir.dt.float32
BF16 = mybir.dt.bfloat16
NP_BF16 = ml_dtypes.bfloat16
AF = mybir.ActivationFunctionType
ALU = mybir.AluOpType

TG = 8                   # transpose blocks per PSUM group (bank = 1024 bf16)
NG = KB // TG            # 4 groups per tile
VW = 512                 # V-expansion width (one f32 PSUM bank)
NV = D // VW             # 8 V-matmuls per tile
SQ_GPS = 2048            # trailing columns of the square pass done on GPSIMD
ACT_Y = (2, 6)           # y-chunks routed via ACT copy-scale + DVE add


def build_nc():
    nc = bass.Bass("TRN2", target_bir_lowering=False, debug=False,
                   num_devices=N_CORES)
    x = nc.declare_dram_parameter("x", [SHARD, D], BF16, isOutput=False)
    u = nc.declare_dram_parameter("u", [128, KB * R], BF16, isOutput=False)
    v = nc.declare_dram_parameter("v", [R, D], BF16, isOutput=False)
    eye = nc.declare_dram_parameter("eye", [PT, PT], BF16, isOutput=False)
    out = nc.declare_dram_parameter("out", [SHARD, D], BF16, isOutput=True)

    with tile.TileContext(nc) as tc:
        with (
            tc.tile_pool(name="singles", bufs=1) as singles,
            tc.tile_pool(name="xin", bufs=4) as xin,
            tc.tile_pool(name="xtp", bufs=3) as xtp,
            tc.tile_pool(name="yout", bufs=3) as yout,
            tc.tile_pool(name="smalls", bufs=4) as smalls,
            tc.tile_pool(name="keeps", bufs=3) as keeps,
            tc.tile_pool(name="scratch", bufs=2) as scratch,
            tc.tile_pool(name="xt_ps", bufs=3, space="PSUM") as xt_ps,
            tc.tile_pool(name="h_ps", bufs=2, space="PSUM") as h_ps,
            tc.tile_pool(name="d_ps", bufs=3, space="PSUM") as d_ps,
        ):
            u_sb = singles.tile([128, KB, R], BF16)
            nc.sync.dma_start(out=u_sb, in_=u.rearrange("p (k r) -> p k r", r=R))
            v_sb = singles.tile([R, D], BF16)
            nc.sync.dma_start(out=v_sb, in_=v[:, :])
            eye_sb = singles.tile([PT, PT], BF16)
            nc.sync.dma_start(out=eye_sb, in_=eye[:, :])
            eps_sb = singles.tile([128, 1], F32)
            nc.vector.memset(eps_sb, EPS)

            SQ_ACT = D - SQ_GPS
            prev = None  # (hs_sb, rstd, x_sb, t0) of tile i-1

            def emit_expand(hs_sb, rstd, x_sb, t0):
                y_sb = yout.tile([PT, D], BF16)
                for n in range(NV):
                    dps = d_ps.tile([PT, VW], F32, tag="d")
                    nc.tensor.matmul(out=dps, lhsT=hs_sb,
                                     rhs=v_sb[:, n * VW:(n + 1) * VW],
                                     start=True, stop=True)
                    ysl = y_sb[:, n * VW:(n + 1) * VW]
                    xsl = x_sb[:, n * VW:(n + 1) * VW]
                    if n in ACT_Y:
                        dsb = scratch.tile([PT, VW], BF16, tag="dsb")
                        nc.scalar.activation(out=dsb, in_=dps, func=AF.Copy,
                                             scale=rstd)
                        nc.vector.tensor_add(out=ysl, in0=dsb, in1=xsl)
                    else:
                        nc.vector.scalar_tensor_tensor(
                            out=ysl, in0=dps, scalar=rstd, in1=xsl,
                            op0=ALU.mult, op1=ALU.add)
                    if n == NV // 2 - 1:
                        nc.gpsimd.dma_start(out=out[t0:t0 + PT, :D // 2],
                                            in_=y_sb[:, :D // 2])
                nc.gpsimd.dma_start(out=out[t0:t0 + PT, D // 2:],
                                    in_=y_sb[:, D // 2:])

            for it in range(NT):
                t0 = it * PT
                x_sb = xin.tile([PT, D], BF16, tag="x_sb")
                nc.sync.dma_start(out=x_sb, in_=x[t0:t0 + PT, :])

                # RMS stats: sumsq -> sqrt -> reciprocal -> rstd (fp32)
                sumsq = smalls.tile([PT, 1], F32, tag="sumsq")
                sq_a = scratch.tile([PT, D], BF16, tag="sq_a")
                nc.scalar.activation(out=sq_a, in_=x_sb, func=AF.Square,
                                     accum_out=sumsq)
                std = smalls.tile([PT, 1], F32, tag="std")
                nc.scalar.activation(out=std, in_=sumsq, func=AF.Sqrt,
                                     bias=eps_sb, scale=1.0 / D)
                rstd = keeps.tile([PT, 1], F32, tag="rstd")
                nc.vector.reciprocal(out=rstd, in_=std)

                # PE transpose: xT_j = x_j.T @ I (bf16 psum) -> SBUF
                xt_sb = xtp.tile([128, KB * PT], BF16)
                h_psum = h_ps.tile([R, PT], F32, tag="h")
                for g in range(NG):
                    tp = xt_ps.tile([128, TG * PT], BF16, tag="tp")
                    for q in range(TG):
                        j = g * TG + q
                        nc.tensor.matmul(
                            out=tp[:, q * PT:(q + 1) * PT],
                            lhsT=x_sb[:, j * 128:(j + 1) * 128],
                            rhs=eye_sb, is_transpose=True,
                            start=True, stop=True)
                    dst = xt_sb[:, g * TG * PT:(g + 1) * TG * PT]
                    if g < 1:
                        nc.scalar.copy(out=dst, in_=tp)
                    else:
                        nc.vector.tensor_copy(out=dst, in_=tp)

                    # hT += U2_g.T @ xT_g for this group's TG blocks
                    for q in range(TG):
                        j = g * TG + q
                        nc.tensor.matmul(
                            out=h_psum,
                            lhsT=u_sb[:, j, :],
                            rhs=xt_sb[:, j * PT:(j + 1) * PT],
                            start=(j == 0), stop=(j == KB - 1))

                hs_sb = keeps.tile([R, PT], BF16, tag="hs")
                nc.vector.tensor_copy(out=hs_sb, in_=h_psum)

                # V-expansion + y of the PREVIOUS tile (PE never waits on
                # the delta-bank drain: those y-ops already completed while
                # this tile's transposes/U-matmuls ran).
                if prev is not None:
                    emit_expand(*prev)
                prev = (hs_sb, rstd, x_sb, t0)

            emit_expand(*prev)
    return nc


def _rank_mask_np(S):
    s_abs = np.abs(S)
    cum = np.cumsum(s_abs) / max(float(s_abs.sum()), 1e-8)
    hit = cum >= ENERGY_THRESHOLD
    r = int(np.argmax(hit)) + 1 if hit.any() else S.shape[0]
    return (np.arange(S.shape[0]) < r).astype(S.dtype)


def make_in_maps(x, U, S, V, norm_weight, gamma):
    S = np.asarray(S, dtype=np.float32)
    keep = _rank_mask_np(S)
    U2 = (np.asarray(norm_weight, dtype=np.float32)[:, None]
          * np.asarray(U, dtype=np.float32)
          * (S * keep)[None, :]).astype(NP_BF16)
    U2 = np.ascontiguousarray(
        U2.reshape(KB, 128, R).transpose(1, 0, 2).reshape(128, KB * R))
    V2 = (np.asarray(V, dtype=np.float32)
          * np.asarray(gamma, dtype=np.float32)[None, :]).astype(NP_BF16)
    eye = np.eye(PT, dtype=NP_BF16)
    xf = np.ascontiguousarray(
        np.asarray(x, dtype=np.float32).reshape(TOK, D)).astype(NP_BF16)
    shards = np.split(xf, N_CORES, axis=0)
    return [{"x": s, "u": U2, "v": V2, "eye": eye} for s in shards]


_CACHED_NC = None


def run(x, U, S, V, norm_weight, gamma, trace=False, **kw):
    global _CACHED_NC
    if _CACHED_NC is None:
        _CACHED_NC = build_nc()
    in_maps = make_in_maps(x, U, S, V, norm_weight, gamma)
    res = run_bass_kernel_spmd(_CACHED_NC, in_maps,
                               core_ids=list(range(N_CORES)), trace=trace,
                               **kw)
    outs = [np.asarray(res.results[i]["out"]) for i in range(N_CORES)]
    y = np.concatenate(outs, axis=0).reshape(B, T, D).astype(np.float32)
    return y, res


def kernel(x, U, S, V, norm_weight, gamma):
    y, _ = run(x, U, S, V, norm_weight, gamma, trace=False)
    return y


# revision 21
# speedup vs baseline: 1.1875x; 1.0227x over previous
"""AdaptiveRankLinear on Trainium2, 8-core data-parallel Bass/Tile kernel.

Computes  y = x + gamma * (((rmsnorm(x) * norm_weight) @ U) * (S*keep)) @ V
with keep = adaptive-rank mask from the singular-value energy of S.

Sharding: x is flattened to [8192, 4096] tokens and split into 8 shards of
1024 tokens (one per NeuronCore); U/S/V/norm_weight/gamma are tiny and
replicated (folded host-side into two small matrices).

v6: bf16 I/O; rstd deferred to the fused (delta*rstd)+x output op; square
pass split ACT/GPSIMD; xT evacs split ACT/DVE (DVE gets 2x on bf16 PSUM);
y-ops split DVE-direct / ACT-copy-scale+DVE-add; V-expansion emitted one
tile late so PE never stalls on the delta PSUM bank rotation.

Per-core device pipeline (per 128-token tile):
  DMA x(bf16) -> ACT+GPS square halves (accum) -> add/sqrt/recip -> rstd
  PE transpose x_j.T @ I (bf16 psum, 8 blocks/bank) -> ACT/DVE evac
  PE hT += U2_j.T @ xT_j (f32 psum) -> DVE evac bf16
  [one tile later] PE delta = hT.T @ V2 (f32 psum)
  DVE/ACT y = delta*rstd + x -> DMA out (bf16).
"""
import ml_dtypes
import numpy as np

import concourse.bass as bass
import concourse.tile as tile
from concourse import mybir
from concourse.bass_utils import run_bass_kernel_spmd
from concourse.vector_clock import ScopedClock

# ----------------------------------------------------------------------------
# Workaround: this container's walrus accepts at most ONE sync wait per
# instruction, while Tile's sem-assigner can attach several.  Split extras
# into engine-local no-ops placed immediately before the over-waited
# instruction; same for the kernel-tail drain.
# ----------------------------------------------------------------------------
_MAXW = 1


def _split_bb_waits(nc, bb):
    insts = list(bb.instructions)
    out = []
    changed = False
    for inst in insts:
        si = inst.sync_info
        if si is not None and len(si.on_wait) > _MAXW:
            changed = True
            waits = list(si.on_wait)
            extra, keep = waits[:-_MAXW], waits[-_MAXW:]
            for k, w in enumerate(extra):
                nop = mybir.InstNoOp(name=f"{inst.name}_wsplit{k}", ins=[],
                                     outs=[])
                nop.engine = inst.engine
                nop.sync_info = mybir.SyncInfo(on_wait=[w], on_update=[])
                nc.register_instruction(nop, overwrite=True)
                out.append(nop)
            inst.sync_info = mybir.SyncInfo(on_wait=keep,
                                            on_update=list(si.on_update))
        out.append(inst)
    if changed:
        bb.instructions = out


def _patched_drain_and_barrier(self, tick_clock, wait_clock):
    for f in self.nc.m.functions:
        for bb in f.blocks:
            _split_bb_waits(self.nc, bb)

    drain_inst = self.nc.sync.drain()
    wait_clock.add_sem_waits(
        drain_inst.ins, ScopedClock({None: tick_clock.global_clock})
    )
    si = drain_inst.ins.sync_info
    if si is not None and len(si.on_wait) > _MAXW:
        waits = list(si.on_wait)
        drain_inst.ins.sync_info = mybir.SyncInfo(
            on_wait=waits[:_MAXW], on_update=list(si.on_update)
        )
        rest = waits[_MAXW:]
        for i in range(0, len(rest), _MAXW):
            nop = self.nc.sync.nop(nofuse=True, hint="drain_wait_spill")
            nop.ins.sync_info = mybir.SyncInfo(
                on_wait=rest[i:i + _MAXW], on_update=[]
            )

    self.nc.all_engine_barrier()
    assert self.sems is not None
    popped = self.nc._tile_sem_poison_stack.pop()
    assert popped is self._sem_poison
    self.nc.clear_and_free_semaphores(list(self.sems.allocated().values()))
    self.nc.all_engine_barrier()


tile.TileContext._drain_and_barrier = _patched_drain_and_barrier

# ----------------------------------------------------------------------------
# Problem constants (hardcoded; kernel.py must be self-contained).
# ----------------------------------------------------------------------------
N_CORES = 8
B, T, D = 4, 2048, 4096
TOK = B * T              # 8192
R = 16
SHARD = TOK // N_CORES   # 1024
PT = 128                 # tokens per tile
NT = SHARD // PT         # 8
KB = D // 128            # 32 contraction blocks
EPS = 1e-6
ENERGY_THRESHOLD = 0.95
F32 = mybir.dt.float32
BF16 = mybir.dt.bfloat16
NP_BF16 = ml_dtypes.bfloat16
AF = mybir.ActivationFunctionType
ALU = mybir.AluOpType

TG = 8                   # transpose blocks per PSUM group (bank = 1024 bf16)
NG = KB // TG            # 4 groups per tile
VW = 512                 # V-expansion width (one f32 PSUM bank)
NV = D // VW             # 8 V-matmuls per tile
ACT_Y = (2, 6)           # y-chunks routed via ACT copy-scale + DVE add
RP = 32                  # U ranks padded to one 32-col PE strip
NSTRIP = 4               # concurrent col-tiled U strips


def build_nc():
    nc = bass.Bass("TRN2", target_bir_lowering=False, debug=False,
                   num_devices=N_CORES)
    x = nc.declare_dram_parameter("x", [SHARD, D], BF16, isOutput=False)
    u = nc.declare_dram_parameter("u", [128, KB * RP], BF16, isOutput=False)
    v = nc.declare_dram_parameter("v", [128, D], BF16, isOutput=False)
    eye = nc.declare_dram_parameter("eye", [PT, PT], BF16, isOutput=False)
    out = nc.declare_dram_parameter("out", [SHARD, D], BF16, isOutput=True)

    with tile.TileContext(nc) as tc:
        with (
            tc.tile_pool(name="singles", bufs=1) as singles,
            tc.tile_pool(name="xin", bufs=4) as xin,
            tc.tile_pool(name="xtp", bufs=3) as xtp,
            tc.tile_pool(name="yout", bufs=3) as yout,
            tc.tile_pool(name="smalls", bufs=4) as smalls,
            tc.tile_pool(name="keeps", bufs=3) as keeps,
            tc.tile_pool(name="scratch", bufs=2) as scratch,
            tc.tile_pool(name="xt_ps", bufs=3, space="PSUM") as xt_ps,
            tc.tile_pool(name="h_ps", bufs=2, space="PSUM") as h_ps,
            tc.tile_pool(name="d_ps", bufs=3, space="PSUM") as d_ps,
        ):
            u_sb = singles.tile([128, KB, RP], BF16)
            nc.sync.dma_start(out=u_sb, in_=u.rearrange("p (k r) -> p k r", r=RP))
            v_sb = singles.tile([128, D], BF16)
            nc.sync.dma_start(out=v_sb, in_=v[:, :])
            eye_sb = singles.tile([PT, PT], BF16)
            nc.sync.dma_start(out=eye_sb, in_=eye[:, :])
            eps_sb = singles.tile([128, 1], F32)
            nc.vector.memset(eps_sb, EPS)

            prev = None  # (hs_sb, rstd, x_sb, t0) of tile i-1

            def emit_expand(hs_sb, rstd, x_sb, t0):
                y_sb = yout.tile([PT, D], BF16)
                for n in range(NV):
                    dps = d_ps.tile([PT, VW], F32, tag="d")
                    nc.tensor.matmul(out=dps, lhsT=hs_sb,
                                     rhs=v_sb[:, n * VW:(n + 1) * VW],
                                     start=True, stop=True)
                    ysl = y_sb[:, n * VW:(n + 1) * VW]
                    xsl = x_sb[:, n * VW:(n + 1) * VW]
                    if n in ACT_Y:
                        dsb = scratch.tile([PT, VW], BF16, tag="dsb")
                        nc.scalar.activation(out=dsb, in_=dps, func=AF.Copy,
                                             scale=rstd)
                        nc.vector.tensor_add(out=ysl, in0=dsb, in1=xsl)
                    else:
                        nc.vector.scalar_tensor_tensor(
                            out=ysl, in0=dps, scalar=rstd, in1=xsl,
                            op0=ALU.mult, op1=ALU.add)
                    if n == NV // 2 - 1:
                        nc.gpsimd.dma_start(out=out[t0:t0 + PT, :D // 2],
                                            in_=y_sb[:, :D // 2])
                nc.gpsimd.dma_start(out=out[t0:t0 + PT, D // 2:],
                                    in_=y_sb[:, D // 2:])

            for it in range(NT):
                t0 = it * PT
                x_sb = xin.tile([PT, D], BF16, tag="x_sb")
                nc.sync.dma_start(out=x_sb, in_=x[t0:t0 + PT, :])

                # RMS stats: sumsq -> sqrt -> reciprocal -> rstd (fp32)
                sumsq = smalls.tile([PT, 1], F32, tag="sumsq")
                sq_a = scratch.tile([PT, D], BF16, tag="sq_a")
                nc.scalar.activation(out=sq_a, in_=x_sb, func=AF.Square,
                                     accum_out=sumsq)
                std = smalls.tile([PT, 1], F32, tag="std")
                nc.scalar.activation(out=std, in_=sumsq, func=AF.Sqrt,
                                     bias=eps_sb, scale=1.0 / D)
                rstd = keeps.tile([PT, 1], F32, tag="rstd")
                nc.vector.reciprocal(out=rstd, in_=std)

                # PE transpose: xT_j = x_j.T @ I (bf16 psum) -> SBUF
                xt_sb = xtp.tile([128, KB * PT], BF16)
                h_psum = h_ps.tile([128, PT], F32, tag="h")
                for g in range(NG):
                    tp = xt_ps.tile([128, TG * PT], BF16, tag="tp")
                    for q in range(TG):
                        j = g * TG + q
                        nc.tensor.matmul(
                            out=tp[:, q * PT:(q + 1) * PT],
                            lhsT=x_sb[:, j * 128:(j + 1) * 128],
                            rhs=eye_sb, is_transpose=True,
                            start=True, stop=True)
                    dst = xt_sb[:, g * TG * PT:(g + 1) * TG * PT]
                    if g < 1:
                        nc.scalar.copy(out=dst, in_=tp)
                    else:
                        nc.vector.tensor_copy(out=dst, in_=tp)

                    # h strip (j%4) += U2pad_j.T @ xT_j — 4 col-tiled strips
                    # run concurrently in distinct 32-column PE groups.
                    for q in range(TG):
                        j = g * TG + q
                        c = j % NSTRIP
                        nc.tensor.matmul(
                            out=h_psum[32 * c:32 * (c + 1), :],
                            lhsT=u_sb[:, j, :],
                            rhs=xt_sb[:, j * PT:(j + 1) * PT],
                            start=(j // NSTRIP == 0),
                            stop=(j // NSTRIP == KB // NSTRIP - 1),
                            tile_position=(0, 32 * c),
                            skip_group_check=True)

                hs_sb = keeps.tile([128, PT], BF16, tag="hs")
                nc.vector.tensor_copy(out=hs_sb, in_=h_psum)

                # V-expansion + y of the PREVIOUS tile (PE never waits on
                # the delta-bank drain: those y-ops already completed while
                # this tile's transposes/U-matmuls ran).
                if prev is not None:
                    emit_expand(*prev)
                prev = (hs_sb, rstd, x_sb, t0)

            emit_expand(*prev)
    return nc


def _rank_mask_np(S):
    s_abs = np.abs(S)
    cum = np.cumsum(s_abs) / max(float(s_abs.sum()), 1e-8)
    hit = cum >= ENERGY_THRESHOLD
    r = int(np.argmax(hit)) + 1 if hit.any() else S.shape[0]
    return (np.arange(S.shape[0]) < r).astype(S.dtype)


def make_in_maps(x, U, S, V, norm_weight, gamma):
    S = np.asarray(S, dtype=np.float32)
    keep = _rank_mask_np(S)
    U2 = (np.asarray(norm_weight, dtype=np.float32)[:, None]
          * np.asarray(U, dtype=np.float32)
          * (S * keep)[None, :]).astype(NP_BF16)
    U2p = np.zeros((D, RP), dtype=NP_BF16)
    U2p[:, :R] = U2
    U2p = np.ascontiguousarray(
        U2p.reshape(KB, 128, RP).transpose(1, 0, 2).reshape(128, KB * RP))
    V2 = (np.asarray(V, dtype=np.float32)
          * np.asarray(gamma, dtype=np.float32)[None, :]).astype(NP_BF16)
    V2r = np.zeros((128, D), dtype=NP_BF16)
    for c in range(NSTRIP):
        V2r[32 * c:32 * c + R, :] = V2
    eye = np.eye(PT, dtype=NP_BF16)
    xf = np.ascontiguousarray(
        np.asarray(x, dtype=np.float32).reshape(TOK, D)).astype(NP_BF16)
    shards = np.split(xf, N_CORES, axis=0)
    return [{"x": s, "u": U2p, "v": V2r, "eye": eye} for s in shards]


_CACHED_NC = None


def run(x, U, S, V, norm_weight, gamma, trace=False, **kw):
    global _CACHED_NC
    if _CACHED_NC is None:
        _CACHED_NC = build_nc()
    in_maps = make_in_maps(x, U, S, V, norm_weight, gamma)
    res = run_bass_kernel_spmd(_CACHED_NC, in_maps,
                               core_ids=list(range(N_CORES)), trace=trace,
                               **kw)
    outs = [np.asarray(res.results[i]["out"]) for i in range(N_CORES)]
    y = np.concatenate(outs, axis=0).reshape(B, T, D).astype(np.float32)
    return y, res


def kernel(x, U, S, V, norm_weight, gamma):
    y, _ = run(x, U, S, V, norm_weight, gamma, trace=False)
    return y
